# revision 11
# baseline (speedup 1.0000x reference)
"""Trainium2 Bass kernel for CrossAttentionFusion.

Math (kv seq_len == 1 collapses attention to two chained linear layers):
    eeg_att = ecg @ (Wo1 @ Wv1).T + (bv1 @ Wo1.T + bo1)
    eeg_out = LN(eeg + eeg_att) * g1 + beta1
    ecg_att = eeg @ (Wo2 @ Wv2).T + (bv2 @ Wo2.T + bo2)
    ecg_out = LN(ecg + ecg_att) * g2 + beta2
    out     = eeg_out @ WfL.T + ecg_out @ WfR.T + bf     (Wf = [WfL | WfR])

g/beta are folded into the fusion weights on the host:
    out = z1 @ (WfL*g1).T + z2 @ (WfR*g2).T + (bf + beta1@WfL.T + beta2@WfR.T)
where z = (a - mean(a)) * rsqrt(var(a) + eps) is the bare standardization.

Per 128-row block (all matmuls bf16 with f32 PSUM accumulate):
  gpsimd cast-DMA loads x bf16 straight -> SB->SB DMA_TRANSPOSE (sync
  engine) makes x.T -> attention matmul (rows-on-partition PSUM) ->
  residual + LN on DVE (Rsqrt on ACT) -> z transposed via SB->SB
  DMA_TRANSPOSE on the scalar engine -> fused matmul -> f32 store.
  The fused matmul for block j is issued after the attention matmuls of
  block j+1 so the PE never waits on the LN/transpose chain.

Weights are host-packed to [128, 8*D] so they load as plain 2-D DMAs
(the 3-D rearrange pattern lowers to a slow serial DIRECT2D transfer).

Sharding: pure data parallel over the batch dim across 8 NeuronCores.
"""

import numpy as np
import ml_dtypes

import concourse.bass as bass
import concourse.mybir as mybir
import concourse.tile as tile
from concourse import bacc

B, D = 32768, 1024
N_CORES = 8
ROWS_PER_CORE = B // N_CORES
EPS = 1e-5
F32 = mybir.dt.float32
BF16 = mybir.dt.bfloat16
BLK = 128  # row block (psum partition tile)
ts = bass.ts
AF = mybir.ActivationFunctionType
ALU = mybir.AluOpType


def build_program(n_rows=ROWS_PER_CORE, use_b1=False, use_b2=False, use_bf=False):
    nc = bacc.Bacc("TRN2", target_bir_lowering=False, debug=False)
    x1 = nc.dram_tensor("x1", (n_rows, D), F32, kind="ExternalInput").ap()
    x2 = nc.dram_tensor("x2", (n_rows, D), F32, kind="ExternalInput").ap()
    # host-packed weights: w[p, c*D + n] = W.T[c*128 + p, n]
    w1t = nc.dram_tensor("w1t", (128, 8 * D), BF16, kind="ExternalInput").ap()
    w2t = nc.dram_tensor("w2t", (128, 8 * D), BF16, kind="ExternalInput").ap()
    wflt = nc.dram_tensor("wflt", (128, 8 * D), BF16, kind="ExternalInput").ap()
    wfrt = nc.dram_tensor("wfrt", (128, 8 * D), BF16, kind="ExternalInput").ap()
    b1 = nc.dram_tensor("b1", (D,), F32, kind="ExternalInput").ap() if use_b1 else None
    b2 = nc.dram_tensor("b2", (D,), F32, kind="ExternalInput").ap() if use_b2 else None
    bfp = (
        nc.dram_tensor("bfp", (D,), F32, kind="ExternalInput").ap() if use_bf else None
    )
    out = nc.dram_tensor("out", (n_rows, D), F32, kind="ExternalOutput").ap()

    n_blocks = n_rows // BLK

    with tile.TileContext(nc) as tc:
        from contextlib import ExitStack

        with ExitStack() as ctx:
            consts = ctx.enter_context(tc.tile_pool(name="consts", bufs=1))
            xb_pool = ctx.enter_context(tc.tile_pool(name="xb", bufs=6))
            xt_pool = ctx.enter_context(tc.tile_pool(name="xt", bufs=5))
            work = ctx.enter_context(tc.tile_pool(name="work", bufs=4))
            zpool = ctx.enter_context(tc.tile_pool(name="z", bufs=4))
            ztpool = ctx.enter_context(tc.tile_pool(name="zt", bufs=6))
            opool = ctx.enter_context(tc.tile_pool(name="o", bufs=3))
            stats = ctx.enter_context(tc.tile_pool(name="stats", bufs=6))
            fences = ctx.enter_context(tc.tile_pool(name="fences", bufs=4))
            psum_mm = ctx.enter_context(
                tc.tile_pool(name="psum_mm", bufs=2, space="PSUM")
            )
            psum_o = ctx.enter_context(
                tc.tile_pool(name="psum_o", bufs=2, space="PSUM")
            )

            # --- constants / weights (loaded once, plain 2-D DMAs) ---
            w1t_sb = consts.tile([128, 8, D], BF16)
            nc.gpsimd.dma_start(w1t_sb, w1t)
            w2t_sb = consts.tile([128, 8, D], BF16)
            nc.gpsimd.dma_start(w2t_sb, w2t)
            wflt_sb = consts.tile([128, 8, D], BF16)
            nc.gpsimd.dma_start(wflt_sb, wflt)
            wfrt_sb = consts.tile([128, 8, D], BF16)
            nc.gpsimd.dma_start(wfrt_sb, wfrt)
            eps_sb = consts.tile([128, 1], F32)
            nc.vector.memset(eps_sb, EPS)
            b1_sb = b2_sb = bf_sb = None
            if use_b1:
                b1_sb = consts.tile([128, D], F32)
                nc.gpsimd.dma_start(b1_sb, b1.partition_broadcast(128))
            if use_b2:
                b2_sb = consts.tile([128, D], F32)
                nc.gpsimd.dma_start(b2_sb, b2.partition_broadcast(128))
            if use_bf:
                bf_sb = consts.tile([128, D], F32)
                nc.gpsimd.dma_start(bf_sb, bfp.partition_broadcast(128))

            # The tile framework does not track the reads/writes of
            # InstDmaTransposeAnt, so every DMA_TRANSPOSE needs manual
            # RAW edges (transpose after source write, consumer after
            # transpose) and WAR edges (buffer reuse after transpose read).
            def dep(a, b, reason=""):
                ia = a.ins if hasattr(a, "ins") else a
                ib = b.ins if hasattr(b, "ins") else b
                tile.add_dep_helper(ia, ib, reason=reason)
            xb_tr_read = {}  # (br, slot) -> transpose inst that read this xb buf
            xt_mm_read = {}  # (br, slot) -> last matmul inst that read this xt buf
            z_tr_read = {}  # slot -> transpose inst that read this z buf
            zt_mm_read = {}  # slot -> last matmul inst that read this zt buf
            z_ctr = 0
            zt_ctr = 0
            XB_BUFS, XT_BUFS, Z_BUFS, ZT_BUFS = 6, 5, 4, 6

            prev = None  # z-transposes of block j-1, consumed by fused matmul
            pending_stores = []  # (row, o) flushed with lag 2

            for j in range(n_blocks + 1):
                # flush old output stores first so they never head-of-line
                # block the x transposes on the sync queue
                while len(pending_stores) > 2:
                    r0, o0 = pending_stores.pop(0)
                    nc.gpsimd.dma_start(out[r0 : r0 + BLK, :], o0)

                ps1 = ps2 = None
                x1b = x2b = None
                if j < n_blocks:
                    r = j * BLK
                    # straight bf16 copies (cast during DMA) - residual stream
                    x1b = xb_pool.tile([128, D], BF16, name="x1b")
                    i_c1 = nc.gpsimd.dma_start(x1b, x1[r : r + BLK, :])
                    x2b = xb_pool.tile([128, D], BF16, name="x2b")
                    i_c2 = nc.gpsimd.dma_start(x2b, x2[r : r + BLK, :])
                    for br, i_c in ((0, i_c1), (1, i_c2)):
                        old = xb_tr_read.get((br, j % XB_BUFS))
                        if old is not None:
                            dep(i_c, old, reason="xb reuse waits on transpose fence")
                    # transposed copies: [din_chunk=128, c, rows=128]
                    x1t = xt_pool.tile([128, 8, BLK], BF16, name="x1t")
                    i_t1 = nc.sync.dma_start(x1t, x1b, transpose=True)
                    dep(i_t1, i_c1, reason="x1t transpose waits on cast write")
                    x2t = xt_pool.tile([128, 8, BLK], BF16, name="x2t")
                    i_t2 = nc.sync.dma_start(x2t, x2b, transpose=True)
                    dep(i_t2, i_c2, reason="x2t transpose waits on cast write")
                    # fence: tiny tracked DMA behind both transposes on the
                    # sync engine; its completion implies both landed
                    xf = fences.tile([1, 4], BF16, name="xf")
                    i_xf = nc.sync.dma_start(xf, x2t[127:128, 7:8, BLK - 4 : BLK])
                    dep(i_xf, i_t2, reason="fence after transpose dispatch")
                    xb_tr_read[(0, j % XB_BUFS)] = i_xf
                    xb_tr_read[(1, j % XB_BUFS)] = i_xf
                    for br, i_t in ((0, i_t1), (1, i_t2)):
                        old = xt_mm_read.get((br, j % XT_BUFS))
                        if old is not None:
                            dep(i_t, old, reason="xt reuse waits on matmul read")
                    # attended = x_other @ W.T    [128 rows, 1024]
                    ps1 = psum_mm.tile([128, D], F32, name="ps")
                    ps2 = psum_mm.tile([128, D], F32, name="ps")
                    for ps, xt_op, wt, i_t, br_xt in (
                        (ps1, x2t, w1t_sb, i_t2, 1),
                        (ps2, x1t, w2t_sb, i_t1, 0),
                    ):
                        for c in range(8):
                            lhsT = xt_op[:, c, :]
                            m0 = nc.tensor.matmul(
                                ps[:, 0:512],
                                lhsT,
                                wt[:, c, 0:512],
                                start=(c == 0),
                                stop=(c == 7),
                            )
                            m1 = nc.tensor.matmul(
                                ps[:, 512:1024],
                                lhsT,
                                wt[:, c, 512:1024],
                                start=(c == 0),
                                stop=(c == 7),
                            )
                            if c == 0:
                                dep(m0, i_xf, reason="attn matmul waits on x.T fence")
                                dep(m1, i_xf, reason="attn matmul waits on x.T fence")
                        xt_mm_read[(br_xt, j % XT_BUFS)] = m1

                # fused matmul of the previous block (PE queue: sits after
                # this block's attention matmuls, so its z's are long ready)
                if prev is not None:
                    z1t, z2t, rp, i_zt1, i_zt2, slot1, slot2 = prev
                    po = psum_o.tile([128, D], F32, name="po")
                    for br in range(2):
                        zt = z1t if br == 0 else z2t
                        i_zt = i_zt1 if br == 0 else i_zt2
                        wt = wflt_sb if br == 0 else wfrt_sb
                        for c in range(8):
                            lhsT = zt[:, c, :]
                            m0 = nc.tensor.matmul(
                                po[:, 0:512],
                                lhsT,
                                wt[:, c, 0:512],
                                start=(br == 0 and c == 0),
                                stop=(br == 1 and c == 7),
                            )
                            m1 = nc.tensor.matmul(
                                po[:, 512:1024],
                                lhsT,
                                wt[:, c, 512:1024],
                                start=(br == 0 and c == 0),
                                stop=(br == 1 and c == 7),
                            )
                            if c == 0:
                                dep(m0, i_zt, reason="fused matmul waits z.T fence")
                                dep(m1, i_zt, reason="fused matmul waits z.T fence")
                    zt_mm_read[slot1] = m1
                    zt_mm_read[slot2] = m1
                    o = opool.tile([128, D], F32, name="o")
                    if bf_sb is not None:
                        nc.vector.tensor_add(o, po, bf_sb)
                    else:
                        nc.vector.tensor_copy(o, po)
                    pending_stores.append((rp, o))
                    prev = None

                if j < n_blocks:
                    r = j * BLK
                    zts = []
                    for br in range(2):
                        ps = ps1 if br == 0 else ps2
                        res = x1b if br == 0 else x2b  # residual stream
                        bias_sb = b1_sb if br == 0 else b2_sb
                        # a = residual + attended (+ bias)
                        a = work.tile([128, D], F32, name="a")
                        nc.vector.tensor_add(a, ps, res)
                        if bias_sb is not None:
                            nc.vector.tensor_add(a, a, bias_sb)
                        # layernorm statistics
                        st = stats.tile([128, 2, 6], F32, name="st")
                        nc.vector.bn_stats(st[:, 0, :], a[:, 0:512])
                        nc.vector.bn_stats(st[:, 1, :], a[:, 512:1024])
                        mv = stats.tile([128, 2], F32, name="mv")
                        nc.vector.bn_aggr(mv, st)
                        rstd = stats.tile([128, 1], F32, name="rstd")
                        nc.scalar.activation(rstd, mv[:, 1:2], AF.Sqrt, bias=eps_sb)
                        nc.vector.reciprocal(rstd, rstd)
                        # z = (a - mean) * rstd, cast to bf16
                        z = zpool.tile([128, D], BF16, name="z")
                        i_z = nc.vector.tensor_scalar(
                            z, a, mv[:, 0:1], rstd, op0=ALU.subtract, op1=ALU.mult
                        )
                        old = z_tr_read.get(z_ctr % Z_BUFS)
                        if old is not None:
                            dep(i_z, old, reason="z reuse waits on transpose fence")
                        # transpose z on the sync queue (all transposes share one
                        # engine: the xbar is a single shared resource)
                        zt = ztpool.tile([128, 8, BLK], BF16, name="zt")
                        i_zt = nc.sync.dma_start(zt, z, transpose=True)
                        dep(i_zt, i_z, reason="z.T transpose waits on z write")
                        z_tr_slot = z_ctr % Z_BUFS
                        zt_slot = zt_ctr % ZT_BUFS
                        old = zt_mm_read.get(zt_slot)
                        if old is not None:
                            dep(i_zt, old, reason="zt reuse waits on matmul read")
                        z_ctr += 1
                        zt_ctr += 1
                        zts.append((zt, i_zt, zt_slot, z_tr_slot))
                    zf = fences.tile([1, 4], BF16, name="zf")
                    i_zf = nc.sync.dma_start(
                        zf, zts[1][0][127:128, 7:8, BLK - 4 : BLK]
                    )
                    dep(i_zf, zts[1][1], reason="fence after z.T transpose dispatch")
                    z_tr_read[zts[0][3]] = i_zf
                    z_tr_read[zts[1][3]] = i_zf
                    prev = (
                        zts[0][0],
                        zts[1][0],
                        r,
                        i_zf,
                        i_zf,
                        zts[0][2],
                        zts[1][2],
                    )
            # drain remaining output stores
            for r0, o0 in pending_stores:
                nc.gpsimd.dma_start(out[r0 : r0 + BLK, :], o0)
            pending_stores.clear()
    nc.compile()
    return nc


def _host_prep(Wv1, bv1, Wo1, bo1, Wv2, bv2, Wo2, bo2, g1, beta1, g2, beta2, Wf, bf):
    f32 = np.float32
    bfd = ml_dtypes.bfloat16
    Wv1, Wo1, Wv2, Wo2, Wf = (np.asarray(a, f32) for a in (Wv1, Wo1, Wv2, Wo2, Wf))
    bv1, bo1, bv2, bo2, bf = (np.asarray(a, f32) for a in (bv1, bo1, bv2, bo2, bf))
    g1, beta1, g2, beta2 = (np.asarray(a, f32) for a in (g1, beta1, g2, beta2))

    W1 = Wo1 @ Wv1  # [dout, din]
    W2 = Wo2 @ Wv2
    b1 = bv1 @ Wo1.T + bo1
    b2 = bv2 @ Wo2.T + bo2
    WfL = Wf[:, :D] * g1[None, :]
    WfR = Wf[:, D:] * g2[None, :]
    bfp = bf + beta1 @ Wf[:, :D].T + beta2 @ Wf[:, D:].T

    def pack(Wm):  # [dout, din] -> [128, 8*D] with w[p, c*D+n] = W.T[c*128+p, n]
        wt = np.ascontiguousarray(Wm.T)  # [din, dout]
        return np.ascontiguousarray(
            wt.reshape(8, 128, D).transpose(1, 0, 2).reshape(128, 8 * D)
        ).astype(bfd)

    weights = {
        "w1t": pack(W1),
        "w2t": pack(W2),
        "wflt": pack(WfL),
        "wfrt": pack(WfR),
    }
    use_b1 = bool(np.any(b1 != 0))
    use_b2 = bool(np.any(b2 != 0))
    use_bf = bool(np.any(bfp != 0))
    if use_b1:
        weights["b1"] = b1
    if use_b2:
        weights["b2"] = b2
    if use_bf:
        weights["bfp"] = bfp
    return weights, use_b1, use_b2, use_bf


def kernel(
    eeg_emb,
    ecg_emb,
    Wv1,
    bv1,
    Wo1,
    bo1,
    Wv2,
    bv2,
    Wo2,
    bo2,
    g1,
    beta1,
    g2,
    beta2,
    Wf,
    bf,
    _run_kwargs=None,
):
    from concourse.bass_utils import run_bass_kernel_spmd

    eeg = np.ascontiguousarray(np.asarray(eeg_emb, np.float32))
    ecg = np.ascontiguousarray(np.asarray(ecg_emb, np.float32))
    weights, use_b1, use_b2, use_bf = _host_prep(
        Wv1, bv1, Wo1, bo1, Wv2, bv2, Wo2, bo2, g1, beta1, g2, beta2, Wf, bf
    )
    nc = build_program(ROWS_PER_CORE, use_b1, use_b2, use_bf)
    in_maps = []
    for i in range(N_CORES):
        sl = slice(i * ROWS_PER_CORE, (i + 1) * ROWS_PER_CORE)
        in_maps.append({"x1": eeg[sl], "x2": ecg[sl], **weights})
    res = run_bass_kernel_spmd(
        nc, in_maps, core_ids=list(range(N_CORES)), **(_run_kwargs or {})
    )
    out = np.concatenate([r["out"] for r in res.results], axis=0)
    if _run_kwargs:
        kernel.last_results = res
    return out
